# revision 19
# baseline (speedup 1.0000x reference)
"""BernNet head on 8 Trainium2 NeuronCores.

Math: the model is logits = mean_N( g(L) @ relu(X W1 + b1) ) @ W2 + b2 with
g(L) = sum_i theta_i C(K,i) L^i (I-L)^{K-i}.  Because mean-pooling over nodes
is a linear functional, the whole polynomial filter collapses onto a single
row vector w^T = (1/N) 1^T g(L) = sum_j c_j q_j^T with q_j^T = (1/N) 1^T L^j,
where c_j is the monomial expansion of the Bernstein coefficients.  The kernel
therefore runs a 10-step vector-transpose chain u <- L^T u + c_j q0 instead of
the 20 dense (N,N)@(N,F) feature applies — same function, ~250x fewer FLOPs.

Numerics: the chain vector is mean-dominated, so it is tracked in mean-removed
form u = (a/N) 1 + d.  Row-stochasticity of L gives d' = L^T d + a*eps with
eps = colsum(L)/N - 1/N and the scalar ledger a' = a + c_j, keeping d exactly
zero-sum.  In this basis fp16 storage of L and d reproduces the fp32 reference
to ~1e-6 relative; all per-element products accumulate in fp32 PSUM.

Distribution: batch-parallel SPMD — core b computes batch item b end to end
(Hf_b = relu(X_b W1 + b1) in fp32, then w^T Hf_b W2 + b2); L, weights and the
chain are replicated per core, so no collectives are needed.
"""

import math
import sys

import numpy as np

for _p in ("/opt/trn_rl_repo", "/root/.axon_site/_ro/trn_rl_repo"):
    if _p not in sys.path:
        sys.path.append(_p)

import concourse.bacc as bacc
import concourse.bass as bass
import concourse.tile as tile
from concourse import mybir
from concourse.bass_utils import run_bass_kernel_spmd

F32 = mybir.dt.float32
F16 = mybir.dt.float16
F8 = mybir.dt.float8e4

B, N, F0, HID, OUT, K = 8, 2048, 128, 64, 16, 10
P = 128
NT = N // P  # 16 tiles per matrix dim
INV_N = 1.0 / N
LSC = 2048.0  # fp8 storage scale for L (entries ~5e-4 -> ~1)


def _m2_matrix() -> np.ndarray:
    """[11, 11] constant: theta^T @ M2 = [A_9..A_0, T].

    c_j = C(K,j) sum_i theta_i C(j,i) (-1)^(j-i)  (monomial expansion of the
    Bernstein basis).  A_col = sum_{m=10-col..10} c_m is the scalar mean
    ledger used at chain step `col`; T = sum_m c_m scales the final mean.
    """
    mbt = np.zeros((K + 1, K + 1))
    for i in range(K + 1):
        for j in range(i, K + 1):
            mbt[i, j] = math.comb(K, j) * math.comb(j, i) * (-1) ** (j - i)
    m2 = np.zeros((K + 1, K + 1), np.float32)
    for col in range(K):
        m2[:, col] = mbt[:, K - col :].sum(axis=1)
    m2[:, K] = mbt.sum(axis=1)
    return m2


def _build_program():
    nc = bacc.Bacc("TRN2", target_bir_lowering=False, debug=False, num_devices=B)

    # fpk layout (fp32 smalls): [ th(1) | m2(11) | w2(16) | b2row | pad ]
    FW = 64
    # f16pk layout (fp16): [ w1(64) | b1row(64, partition 0) ]
    F16W = 128
    lpk_d = nc.dram_tensor("lpk", [P, NT * N], F8, kind="ExternalInput").ap()
    fpk_d = nc.dram_tensor("fpk", [P, FW], F32, kind="ExternalInput").ap()
    f16pk_d = nc.dram_tensor("f16pk", [P, F16W], F16, kind="ExternalInput").ap()
    x16_d = nc.dram_tensor("x16", [P, N], F16, kind="ExternalInput").ap()
    out_d = nc.dram_tensor("logits", [OUT, 1], F32, kind="ExternalOutput").ap()

    def ltile(lh8, k, m):
        # stationary L tile (k, m): lhsT[v, w] = L[k*128+v, m*128+w]
        t = lh8[m // 2]
        return t[:, ((m % 2) * NT + k) * P : ((m % 2) * NT + k + 1) * P]

    with tile.TileContext(nc) as tc:
        import contextlib

        with contextlib.ExitStack() as ctx:
            cb = ctx.enter_context(tc.tile_pool(name="cb", bufs=1))
            wb = ctx.enter_context(tc.tile_pool(name="wb", bufs=2))
            pm = ctx.enter_context(tc.tile_pool(name="pm", bufs=2, space="PSUM"))
            pz = ctx.enter_context(tc.tile_pool(name="pz", bufs=2, space="PSUM"))
            pc = ctx.enter_context(tc.tile_pool(name="pc", bufs=4, space="PSUM"))

            # ---- input loads: small packs first (feature/coef start early),
            # then X^T, then 8 L chunks as separate tiles so colsum can
            # stream behind the DMA (chunk i = m-blocks 2i, 2i+1).
            fpk = cb.tile([P, FW], F32, tag="fpk")
            nc.gpsimd.dma_start(out=fpk[:], in_=fpk_d)
            f16pk = cb.tile([P, F16W], F16, tag="f16pk")
            nc.gpsimd.dma_start(out=f16pk[:], in_=f16pk_d)
            x16 = cb.tile([P, N], F16, tag="x16")
            nc.gpsimd.dma_start(out=x16[:], in_=x16_d)
            lh8 = []
            for i in range(8):
                t = cb.tile([P, NT * N // 8], F8, name=f"lh_{i}", tag=f"lh_{i}")
                eng = nc.scalar if i % 2 == 0 else nc.sync
                eng.dma_start(out=t[:], in_=lpk_d[:, bass.ts(i, NT * N // 8)])
                lh8.append(t)

            th = fpk[0 : K + 1, 0:1]
            m2 = fpk[0 : K + 1, 1 : 1 + (K + 1)]
            w2 = fpk[0:HID, 12 : 12 + OUT]
            w1 = f16pk[:, 0:HID]
            b1row16 = f16pk[0:1, HID : HID + HID]
            ones16 = cb.tile([P, 1], F16, tag="ones16")
            nc.vector.memset(ones16[:], 1.0)
            ones16r = cb.tile([1, P], F16, tag="ones16r")
            nc.vector.memset(ones16r[:], 1.0)
            ident1 = cb.tile([1, 1], F32, tag="ident1")
            nc.vector.memset(ident1[:], 1.0)

            onesr_t = cb.tile([1, P], F32, tag="onesr")
            nc.vector.memset(onesr_t[:], 1.0)
            onesr = onesr_t[0:1, 0:P]
            b2row = fpk[0:1, 28 : 28 + OUT]

            # ---- coefficients: coefRow = theta^T @ M2 -> broadcast to 128 rows
            ps_cf = pm.tile([1, K + 1], F32, tag="pm")
            nc.tensor.matmul(ps_cf[:], th, m2, start=True, stop=True)
            cfrow = cb.tile([1, K + 1], F32, tag="cfrow")
            nc.vector.tensor_copy(cfrow[:], ps_cf[:])
            ps_cb = pm.tile([P, K + 1], F32, tag="pm")
            nc.tensor.matmul(ps_cb[:], onesr, cfrow[:], start=True, stop=True)
            coefb = cb.tile([P, K + 1], F32, tag="coefb")
            nc.vector.tensor_copy(coefb[:], ps_cb[:])

            # per-iteration mean-ledger scalars replicated per partition:
            # scale_j = A_j/N, bias_j = -A_j/N (for the colsum->d16_0 fold)
            sc0 = cb.tile([P, 1], F32, tag="sc0")
            nc.vector.tensor_scalar_mul(sc0[:], coefb[:, 0:1], INV_N)
            nb0 = cb.tile([P, 1], F32, tag="nb0")
            nc.vector.tensor_scalar_mul(nb0[:], sc0[:], -LSC)
            nbias = cb.tile([P, 1], F32, tag="nbias")
            nc.vector.memset(nbias[:], -INV_N)

            G = 4  # m-groups per chain step (psum bank + assembly granularity)
            GM = NT // G

            # ---- colsum(L): d16_0 = fp16(A_0*(colsum/N - 1/N)) per group,
            # plus eps = colsum/N - 1/N in fp32 for the later iterations.
            d16 = [wb.tile([P, GM], F16, name=f"d16i0_{g}", tag=f"d16_{g}") for g in range(G)]
            eps = cb.tile([P, NT], F32, tag="eps")
            for g in range(G):
                ps_cs = pc.tile([P, GM], F32, tag="pc")
                for mg in range(GM):
                    m = g * GM + mg
                    for k in range(NT):
                        nc.tensor.matmul(
                            ps_cs[:, mg : mg + 1],
                            ltile(lh8, k, m),
                            ones16[:],
                            start=(k == 0),
                            stop=(k == NT - 1),
                        )
                nc.scalar.activation(
                    d16[g][:], ps_cs[:], mybir.ActivationFunctionType.Identity,
                    bias=nb0[:], scale=sc0[:],
                )
                nc.scalar.activation(
                    eps[:, g * GM : (g + 1) * GM], ps_cs[:],
                    mybir.ActivationFunctionType.Identity,
                    bias=nbias[:], scale=INV_N / LSC,
                )

            # ---- feature side: Hf = relu(X W1 + b1), natural [v, h] layout
            # (X^T arrives pre-transposed; fp16 operands, fp32 PSUM accum)
            hf = cb.tile([P, NT * HID], F32, tag="hf")
            for t in range(NT):
                ps_z = pz.tile([P, HID], F32, tag="pz")
                nc.tensor.matmul(ps_z[:], x16[:, bass.ts(t, P)], w1, start=True, stop=False)
                nc.tensor.matmul(ps_z[:], ones16r[:], b1row16, start=False, stop=True)
                nc.scalar.activation(
                    hf[:, bass.ts(t, HID)], ps_z[:], mybir.ActivationFunctionType.Relu
                )

            # all A_j * eps tiles up front (off the critical path)
            epsa = []
            for it in range(1, K):
                ea = cb.tile([P, NT], F32, tag=f"epsa_{it}")
                nc.vector.tensor_scalar(ea[:], eps[:], coefb[:, it : it + 1], LSC * LSC, mybir.AluOpType.mult, mybir.AluOpType.mult)
                epsa.append(ea)

            # ---- chain: d' = L^T d + A_j * eps   (d zero-sum, fp16 storage)
            dfin = None
            for it in range(1, K):
                ea = epsa[it - 1]
                last = it == K - 1
                d16n = None if last else [
                    wb.tile([P, GM], F16, name=f"d16i{it}_{g}", tag=f"d16_{g}")
                    for g in range(G)
                ]
                if last:
                    dfin = [
                        wb.tile([P, GM], F32, name=f"dfin_{g}", tag=f"dfin_{g}")
                        for g in range(G)
                    ]
                ps_g = [
                    pc.tile([P, GM], F32, name=f"psch{it}_{g}", tag="pc")
                    for g in range(G)
                ]
                for k in range(NT):
                    rhs = d16[k // GM][:, k % GM : k % GM + 1]
                    for m in range(NT):
                        nc.tensor.matmul(
                            ps_g[m // GM][:, m % GM : m % GM + 1],
                            ltile(lh8, k, m),
                            rhs,
                            start=(k == 0),
                            stop=(k == NT - 1),
                        )
                for g in range(G):
                    tgt = dfin[g] if last else d16n[g]
                    if last:
                        # dfin stays in scaled space; descale folds into the
                        # wf activation below
                        nc.vector.tensor_add(
                            tgt[:], ps_g[g][:], ea[:, g * GM : (g + 1) * GM]
                        )
                    else:
                        tmp = wb.tile(
                            [P, GM], F32, name=f"asm{it}_{g}", tag=f"asm_{g}"
                        )
                        nc.vector.tensor_add(
                            tmp[:], ps_g[g][:], ea[:, g * GM : (g + 1) * GM]
                        )
                        nc.vector.tensor_scalar_mul(tgt[:], tmp[:], 1.0 / LSC)
                if not last:
                    d16 = d16n

            # w = (T/N) 1 + d ; s = w^T Hf  (per group, so s-matmuls of group g
            # start as soon as group g's chain output lands)
            tn = cb.tile([P, 1], F32, tag="tn")
            nc.scalar.mul(tn[:], coefb[:, K : K + 1], INV_N)
            wf = cb.tile([P, NT], F32, tag="wf")
            ps_s = pm.tile([1, HID], F32, tag="pm")
            for g in range(G):
                nc.scalar.activation(
                    wf[:, g * GM : (g + 1) * GM], dfin[g][:],
                    mybir.ActivationFunctionType.Identity, bias=tn[:],
                    scale=1.0 / (LSC * LSC),
                )
                for mg in range(GM):
                    t = g * GM + mg
                    nc.tensor.matmul(
                        ps_s[:],
                        wf[:, t : t + 1],
                        hf[:, bass.ts(t, HID)],
                        start=(t == 0),
                        stop=(t == NT - 1),
                    )
            srow = cb.tile([1, HID], F32, tag="srow")
            nc.vector.tensor_copy(srow[:], ps_s[:])
            ps_st = pm.tile([HID, 1], F32, tag="pm")
            nc.tensor.transpose(ps_st[:], srow[:], ident1[:])
            st = cb.tile([HID, 1], F32, tag="st")
            nc.vector.tensor_copy(st[:], ps_st[:])
            ps_o = pm.tile([OUT, 1], F32, tag="pm")
            nc.tensor.matmul(ps_o[:], w2, st[:], start=True, stop=False)
            nc.tensor.matmul(ps_o[:], b2row, onesr[0:1, 0:1], start=False, stop=True)
            outt = cb.tile([OUT, 1], F32, tag="outt")
            nc.vector.tensor_copy(outt[:], ps_o[:])
            nc.gpsimd.dma_start(out=out_d, in_=outt[:])

    nc.compile()
    return nc


_NC_CACHE = {}


def _get_program():
    if "nc" not in _NC_CACHE:
        _NC_CACHE["nc"] = _build_program()
    return _NC_CACHE["nc"]


def _prepare_in_maps(X, L, W1, b1, W2, b2, theta):
    import ml_dtypes
    lpk = (
        (np.ascontiguousarray(L, np.float32) * np.float32(LSC))
        .reshape(NT, P, NT, P)
        .transpose(1, 2, 0, 3)
        .reshape(P, NT * N)
        .astype(ml_dtypes.float8_e4m3)
    )
    fpk = np.zeros((P, 64), np.float32)
    fpk[0 : K + 1, 0] = np.asarray(theta, np.float32)
    fpk[0 : K + 1, 1 : 1 + (K + 1)] = _m2_matrix()
    fpk[0:HID, 12 : 12 + OUT] = np.asarray(W2, np.float32)
    fpk[0, 28 : 28 + OUT] = np.asarray(b2, np.float32)
    f16pk = np.zeros((P, 128), np.float16)
    f16pk[0:F0, 0:HID] = np.asarray(W1, np.float32).astype(np.float16)
    f16pk[0, HID : HID + HID] = np.asarray(b1, np.float32).astype(np.float16)
    common = {"lpk": lpk, "fpk": fpk, "f16pk": f16pk}
    in_maps = []
    for b in range(B):
        x16 = np.ascontiguousarray(
            np.asarray(X[b], np.float32).T.astype(np.float16)
        )
        in_maps.append({**common, "x16": x16})
    return in_maps


def _run(inputs, trace=False):
    nc = _get_program()
    in_maps = _prepare_in_maps(
        inputs["X"], inputs["L"], inputs["W1"], inputs["b1"],
        inputs["W2"], inputs["b2"], inputs["theta"],
    )
    res = run_bass_kernel_spmd(nc, in_maps, list(range(B)), trace=trace)
    out = np.stack([res.results[b]["logits"].reshape(OUT) for b in range(B)])
    return out.astype(np.float32), res


def kernel(**inputs) -> np.ndarray:
    out, _ = _run(inputs, trace=False)
    return out


def kernel_traced(**inputs):
    return _run(inputs, trace=True)


# revision 20
# speedup vs baseline: 1.0103x; 1.0103x over previous
"""BernNet head on 8 Trainium2 NeuronCores.

Math: the model is logits = mean_N( g(L) @ relu(X W1 + b1) ) @ W2 + b2 with
g(L) = sum_i theta_i C(K,i) L^i (I-L)^{K-i}.  Because mean-pooling over nodes
is a linear functional, the whole polynomial filter collapses onto a single
row vector w^T = (1/N) 1^T g(L) = sum_j c_j q_j^T with q_j^T = (1/N) 1^T L^j,
where c_j is the monomial expansion of the Bernstein coefficients.  The kernel
therefore runs a 10-step vector-transpose chain u <- L^T u + c_j q0 instead of
the 20 dense (N,N)@(N,F) feature applies — same function, ~250x fewer FLOPs.

Numerics: the chain vector is mean-dominated, so it is tracked in mean-removed
form u = (a/N) 1 + d.  Row-stochasticity of L gives d' = L^T d + a*eps with
eps = colsum(L)/N - 1/N and the scalar ledger a' = a + c_j, keeping d exactly
zero-sum.  In this basis fp16 storage of L and d reproduces the fp32 reference
to ~1e-6 relative; all per-element products accumulate in fp32 PSUM.

Distribution: batch-parallel SPMD — core b computes batch item b end to end
(Hf_b = relu(X_b W1 + b1) in fp32, then w^T Hf_b W2 + b2); L, weights and the
chain are replicated per core, so no collectives are needed.
"""

import math
import sys

import numpy as np

for _p in ("/opt/trn_rl_repo", "/root/.axon_site/_ro/trn_rl_repo"):
    if _p not in sys.path:
        sys.path.append(_p)

import concourse.bacc as bacc
import concourse.bass as bass
import concourse.tile as tile
from concourse import mybir
from concourse.bass_utils import run_bass_kernel_spmd

F32 = mybir.dt.float32
F16 = mybir.dt.float16
F8 = mybir.dt.float8e4

B, N, F0, HID, OUT, K = 8, 2048, 128, 64, 16, 10
P = 128
NT = N // P  # 16 tiles per matrix dim
INV_N = 1.0 / N
LSC = 2048.0  # fp8 storage scale for L (entries ~5e-4 -> ~1)


def _m2_matrix() -> np.ndarray:
    """[11, 11] constant: theta^T @ M2 = [A_9..A_0, T].

    c_j = C(K,j) sum_i theta_i C(j,i) (-1)^(j-i)  (monomial expansion of the
    Bernstein basis).  A_col = sum_{m=10-col..10} c_m is the scalar mean
    ledger used at chain step `col`; T = sum_m c_m scales the final mean.
    """
    mbt = np.zeros((K + 1, K + 1))
    for i in range(K + 1):
        for j in range(i, K + 1):
            mbt[i, j] = math.comb(K, j) * math.comb(j, i) * (-1) ** (j - i)
    m2 = np.zeros((K + 1, K + 1), np.float32)
    for col in range(K):
        m2[:, col] = mbt[:, K - col :].sum(axis=1)
    m2[:, K] = mbt.sum(axis=1)
    return m2


def _build_program():
    nc = bacc.Bacc("TRN2", target_bir_lowering=False, debug=False, num_devices=B)

    # fpk layout (fp32 smalls): [ th(1) | m2(11) | w2(16) | b2row | pad ]
    FW = 64
    # f16pk layout (fp16): [ w1(64) | b1row(64, partition 0) ]
    F16W = 128
    lpk_d = nc.dram_tensor("lpk", [P, NT * N], F8, kind="ExternalInput").ap()
    fpk_d = nc.dram_tensor("fpk", [P, FW], F32, kind="ExternalInput").ap()
    f16pk_d = nc.dram_tensor("f16pk", [P, F16W], F16, kind="ExternalInput").ap()
    x16_d = nc.dram_tensor("x16", [P, N], F16, kind="ExternalInput").ap()
    out_d = nc.dram_tensor("logits", [OUT, 1], F32, kind="ExternalOutput").ap()

    def ltile(lh8, k, m):
        # stationary L tile (k, m): lhsT[v, w] = L[k*128+v, m*128+w]
        t = lh8[m // 2]
        return t[:, ((m % 2) * NT + k) * P : ((m % 2) * NT + k + 1) * P]

    with tile.TileContext(nc) as tc:
        import contextlib

        with contextlib.ExitStack() as ctx:
            cb = ctx.enter_context(tc.tile_pool(name="cb", bufs=1))
            wb = ctx.enter_context(tc.tile_pool(name="wb", bufs=2))
            pm = ctx.enter_context(tc.tile_pool(name="pm", bufs=2, space="PSUM"))
            pz = ctx.enter_context(tc.tile_pool(name="pz", bufs=2, space="PSUM"))
            pc = ctx.enter_context(tc.tile_pool(name="pc", bufs=4, space="PSUM"))

            # ---- input loads: small packs first (feature/coef start early),
            # then X^T, then 8 L chunks as separate tiles so colsum can
            # stream behind the DMA (chunk i = m-blocks 2i, 2i+1).
            fpk = cb.tile([P, FW], F32, tag="fpk")
            nc.sync.dma_start(out=fpk[:], in_=fpk_d)
            f16pk = cb.tile([P, F16W], F16, tag="f16pk")
            nc.sync.dma_start(out=f16pk[:], in_=f16pk_d)
            x16 = cb.tile([P, N], F16, tag="x16")
            nc.sync.dma_start(out=x16[:], in_=x16_d)
            lh8 = []
            for i in range(8):
                t = cb.tile([P, NT * N // 8], F8, name=f"lh_{i}", tag=f"lh_{i}")
                eng = nc.scalar if i % 2 == 0 else nc.sync
                eng.dma_start(out=t[:], in_=lpk_d[:, bass.ts(i, NT * N // 8)])
                lh8.append(t)

            th = fpk[0 : K + 1, 0:1]
            m2 = fpk[0 : K + 1, 1 : 1 + (K + 1)]
            w2 = fpk[0:HID, 12 : 12 + OUT]
            w1 = f16pk[:, 0:HID]
            b1row16 = f16pk[0:1, HID : HID + HID]
            ones16 = cb.tile([P, 1], F16, tag="ones16")
            nc.vector.memset(ones16[:], 1.0)
            ones16r = cb.tile([1, P], F16, tag="ones16r")
            nc.vector.memset(ones16r[:], 1.0)
            ident1 = cb.tile([1, 1], F32, tag="ident1")
            nc.vector.memset(ident1[:], 1.0)

            onesr_t = cb.tile([1, P], F32, tag="onesr")
            nc.vector.memset(onesr_t[:], 1.0)
            onesr = onesr_t[0:1, 0:P]
            b2row = fpk[0:1, 28 : 28 + OUT]

            # ---- coefficients: coefRow = theta^T @ M2 -> broadcast to 128 rows
            ps_cf = pm.tile([1, K + 1], F32, tag="pm")
            nc.tensor.matmul(ps_cf[:], th, m2, start=True, stop=True)
            cfrow = cb.tile([1, K + 1], F32, tag="cfrow")
            nc.vector.tensor_copy(cfrow[:], ps_cf[:])
            ps_cb = pm.tile([P, K + 1], F32, tag="pm")
            nc.tensor.matmul(ps_cb[:], onesr, cfrow[:], start=True, stop=True)
            coefb = cb.tile([P, K + 1], F32, tag="coefb")
            nc.vector.tensor_copy(coefb[:], ps_cb[:])

            # per-iteration mean-ledger scalars replicated per partition:
            # scale_j = A_j/N, bias_j = -A_j/N (for the colsum->d16_0 fold)
            sc0 = cb.tile([P, 1], F32, tag="sc0")
            nc.vector.tensor_scalar_mul(sc0[:], coefb[:, 0:1], INV_N)
            nb0 = cb.tile([P, 1], F32, tag="nb0")
            nc.vector.tensor_scalar_mul(nb0[:], sc0[:], -LSC)
            nbias = cb.tile([P, 1], F32, tag="nbias")
            nc.vector.memset(nbias[:], -INV_N)

            G = 4  # m-groups per chain step (psum bank + assembly granularity)
            GM = NT // G

            # ---- colsum(L): d16_0 = fp16(A_0*(colsum/N - 1/N)) per group,
            # plus eps = colsum/N - 1/N in fp32 for the later iterations.
            d16 = [wb.tile([P, GM], F16, name=f"d16i0_{g}", tag=f"d16_{g}") for g in range(G)]
            eps = cb.tile([P, NT], F32, tag="eps")
            for g in range(G):
                ps_cs = pc.tile([P, GM], F32, tag="pc")
                for mg in range(GM):
                    m = g * GM + mg
                    for k in range(NT):
                        nc.tensor.matmul(
                            ps_cs[:, mg : mg + 1],
                            ltile(lh8, k, m),
                            ones16[:],
                            start=(k == 0),
                            stop=(k == NT - 1),
                        )
                nc.scalar.activation(
                    d16[g][:], ps_cs[:], mybir.ActivationFunctionType.Identity,
                    bias=nb0[:], scale=sc0[:],
                )
                nc.scalar.activation(
                    eps[:, g * GM : (g + 1) * GM], ps_cs[:],
                    mybir.ActivationFunctionType.Identity,
                    bias=nbias[:], scale=INV_N / LSC,
                )

            # ---- feature side: Hf = relu(X W1 + b1), natural [v, h] layout
            # (X^T arrives pre-transposed; fp16 operands, fp32 PSUM accum)
            hf = cb.tile([P, NT * HID], F32, tag="hf")
            for t in range(NT):
                ps_z = pz.tile([P, HID], F32, tag="pz")
                nc.tensor.matmul(ps_z[:], x16[:, bass.ts(t, P)], w1, start=True, stop=False)
                nc.tensor.matmul(ps_z[:], ones16r[:], b1row16, start=False, stop=True)
                nc.scalar.activation(
                    hf[:, bass.ts(t, HID)], ps_z[:], mybir.ActivationFunctionType.Relu
                )

            # all A_j * eps tiles up front (off the critical path)
            epsa = []
            for it in range(1, K):
                ea = cb.tile([P, NT], F32, tag=f"epsa_{it}")
                nc.vector.tensor_scalar(ea[:], eps[:], coefb[:, it : it + 1], LSC * LSC, mybir.AluOpType.mult, mybir.AluOpType.mult)
                epsa.append(ea)

            # ---- chain: d' = L^T d + A_j * eps   (d zero-sum, fp16 storage)
            dfin = None
            for it in range(1, K):
                ea = epsa[it - 1]
                last = it == K - 1
                d16n = None if last else [
                    wb.tile([P, GM], F16, name=f"d16i{it}_{g}", tag=f"d16_{g}")
                    for g in range(G)
                ]
                if last:
                    dfin = [
                        wb.tile([P, GM], F32, name=f"dfin_{g}", tag=f"dfin_{g}")
                        for g in range(G)
                    ]
                ps_g = [
                    pc.tile([P, GM], F32, name=f"psch{it}_{g}", tag="pc")
                    for g in range(G)
                ]
                for k in range(NT):
                    rhs = d16[k // GM][:, k % GM : k % GM + 1]
                    for m in range(NT):
                        nc.tensor.matmul(
                            ps_g[m // GM][:, m % GM : m % GM + 1],
                            ltile(lh8, k, m),
                            rhs,
                            start=(k == 0),
                            stop=(k == NT - 1),
                        )
                for g in range(G):
                    tgt = dfin[g] if last else d16n[g]
                    if last:
                        # dfin stays in scaled space; descale folds into the
                        # wf activation below
                        nc.vector.tensor_add(
                            tgt[:], ps_g[g][:], ea[:, g * GM : (g + 1) * GM]
                        )
                    else:
                        tmp = wb.tile(
                            [P, GM], F32, name=f"asm{it}_{g}", tag=f"asm_{g}"
                        )
                        nc.vector.tensor_add(
                            tmp[:], ps_g[g][:], ea[:, g * GM : (g + 1) * GM]
                        )
                        nc.vector.tensor_scalar_mul(tgt[:], tmp[:], 1.0 / LSC)
                if not last:
                    d16 = d16n

            # w = (T/N) 1 + d ; s = w^T Hf  (per group, so s-matmuls of group g
            # start as soon as group g's chain output lands)
            tn = cb.tile([P, 1], F32, tag="tn")
            nc.scalar.mul(tn[:], coefb[:, K : K + 1], INV_N)
            wf = cb.tile([P, NT], F32, tag="wf")
            ps_s = pm.tile([1, HID], F32, tag="pm")
            for g in range(G):
                nc.scalar.activation(
                    wf[:, g * GM : (g + 1) * GM], dfin[g][:],
                    mybir.ActivationFunctionType.Identity, bias=tn[:],
                    scale=1.0 / (LSC * LSC),
                )
                for mg in range(GM):
                    t = g * GM + mg
                    nc.tensor.matmul(
                        ps_s[:],
                        wf[:, t : t + 1],
                        hf[:, bass.ts(t, HID)],
                        start=(t == 0),
                        stop=(t == NT - 1),
                    )
            srow = cb.tile([1, HID], F32, tag="srow")
            nc.vector.tensor_copy(srow[:], ps_s[:])
            ps_st = pm.tile([HID, 1], F32, tag="pm")
            nc.tensor.transpose(ps_st[:], srow[:], ident1[:])
            st = cb.tile([HID, 1], F32, tag="st")
            nc.vector.tensor_copy(st[:], ps_st[:])
            ps_o = pm.tile([OUT, 1], F32, tag="pm")
            nc.tensor.matmul(ps_o[:], w2, st[:], start=True, stop=False)
            nc.tensor.matmul(ps_o[:], b2row, onesr[0:1, 0:1], start=False, stop=True)
            outt = cb.tile([OUT, 1], F32, tag="outt")
            nc.vector.tensor_copy(outt[:], ps_o[:])
            nc.gpsimd.dma_start(out=out_d, in_=outt[:])

    nc.compile()
    return nc


_NC_CACHE = {}


def _get_program():
    if "nc" not in _NC_CACHE:
        _NC_CACHE["nc"] = _build_program()
    return _NC_CACHE["nc"]


def _prepare_in_maps(X, L, W1, b1, W2, b2, theta):
    import ml_dtypes
    lpk = (
        (np.ascontiguousarray(L, np.float32) * np.float32(LSC))
        .reshape(NT, P, NT, P)
        .transpose(1, 2, 0, 3)
        .reshape(P, NT * N)
        .astype(ml_dtypes.float8_e4m3)
    )
    fpk = np.zeros((P, 64), np.float32)
    fpk[0 : K + 1, 0] = np.asarray(theta, np.float32)
    fpk[0 : K + 1, 1 : 1 + (K + 1)] = _m2_matrix()
    fpk[0:HID, 12 : 12 + OUT] = np.asarray(W2, np.float32)
    fpk[0, 28 : 28 + OUT] = np.asarray(b2, np.float32)
    f16pk = np.zeros((P, 128), np.float16)
    f16pk[0:F0, 0:HID] = np.asarray(W1, np.float32).astype(np.float16)
    f16pk[0, HID : HID + HID] = np.asarray(b1, np.float32).astype(np.float16)
    common = {"lpk": lpk, "fpk": fpk, "f16pk": f16pk}
    in_maps = []
    for b in range(B):
        x16 = np.ascontiguousarray(
            np.asarray(X[b], np.float32).T.astype(np.float16)
        )
        in_maps.append({**common, "x16": x16})
    return in_maps


def _run(inputs, trace=False):
    nc = _get_program()
    in_maps = _prepare_in_maps(
        inputs["X"], inputs["L"], inputs["W1"], inputs["b1"],
        inputs["W2"], inputs["b2"], inputs["theta"],
    )
    res = run_bass_kernel_spmd(nc, in_maps, list(range(B)), trace=trace)
    out = np.stack([res.results[b]["logits"].reshape(OUT) for b in range(B)])
    return out.astype(np.float32), res


def kernel(**inputs) -> np.ndarray:
    out, _ = _run(inputs, trace=False)
    return out


def kernel_traced(**inputs):
    return _run(inputs, trace=True)


# revision 21
# speedup vs baseline: 1.0205x; 1.0100x over previous
"""BernNet head on 8 Trainium2 NeuronCores.

Math: the model is logits = mean_N( g(L) @ relu(X W1 + b1) ) @ W2 + b2 with
g(L) = sum_i theta_i C(K,i) L^i (I-L)^{K-i}.  Because mean-pooling over nodes
is a linear functional, the whole polynomial filter collapses onto a single
row vector w^T = (1/N) 1^T g(L) = sum_j c_j q_j^T with q_j^T = (1/N) 1^T L^j,
where c_j is the monomial expansion of the Bernstein coefficients.  The kernel
therefore runs a 10-step vector-transpose chain u <- L^T u + c_j q0 instead of
the 20 dense (N,N)@(N,F) feature applies — same function, ~250x fewer FLOPs.

Numerics: the chain vector is mean-dominated, so it is tracked in mean-removed
form u = (a/N) 1 + d.  Row-stochasticity of L gives d' = L^T d + a*eps with
eps = colsum(L)/N - 1/N and the scalar ledger a' = a + c_j, keeping d exactly
zero-sum.  In this basis fp16 storage of L and d reproduces the fp32 reference
to ~1e-6 relative; all per-element products accumulate in fp32 PSUM.

Distribution: batch-parallel SPMD — core b computes batch item b end to end
(Hf_b = relu(X_b W1 + b1) in fp32, then w^T Hf_b W2 + b2); L, weights and the
chain are replicated per core, so no collectives are needed.
"""

import math
import sys

import numpy as np

for _p in ("/opt/trn_rl_repo", "/root/.axon_site/_ro/trn_rl_repo"):
    if _p not in sys.path:
        sys.path.append(_p)

import concourse.bacc as bacc
import concourse.bass as bass
import concourse.tile as tile
from concourse import mybir
from concourse.bass_utils import run_bass_kernel_spmd

F32 = mybir.dt.float32
F16 = mybir.dt.float16
F8 = mybir.dt.float8e4

B, N, F0, HID, OUT, K = 8, 2048, 128, 64, 16, 10
P = 128
NT = N // P  # 16 tiles per matrix dim
INV_N = 1.0 / N
LSC = 2048.0  # fp8 storage scale for L (entries ~5e-4 -> ~1)


def _m2_matrix() -> np.ndarray:
    """[11, 11] constant: theta^T @ M2 = [A_9..A_0, T].

    c_j = C(K,j) sum_i theta_i C(j,i) (-1)^(j-i)  (monomial expansion of the
    Bernstein basis).  A_col = sum_{m=10-col..10} c_m is the scalar mean
    ledger used at chain step `col`; T = sum_m c_m scales the final mean.
    """
    mbt = np.zeros((K + 1, K + 1))
    for i in range(K + 1):
        for j in range(i, K + 1):
            mbt[i, j] = math.comb(K, j) * math.comb(j, i) * (-1) ** (j - i)
    m2 = np.zeros((K + 1, K + 1), np.float32)
    for col in range(K):
        m2[:, col] = mbt[:, K - col :].sum(axis=1)
    m2[:, K] = mbt.sum(axis=1)
    return m2


def _build_program():
    nc = bacc.Bacc("TRN2", target_bir_lowering=False, debug=False, num_devices=B)

    # fpk layout (fp32 smalls): [ th(1) | m2(11) | w2(16) | b2row | pad ]
    FW = 64
    # f16pk layout (fp16): [ w1(64) | b1row(64, partition 0) ]
    F16W = 128
    lpk_d = nc.dram_tensor("lpk", [P, NT * N], F8, kind="ExternalInput").ap()
    fpk_d = nc.dram_tensor("fpk", [P, FW], F32, kind="ExternalInput").ap()
    f16pk_d = nc.dram_tensor("f16pk", [P, F16W], F16, kind="ExternalInput").ap()
    x16_d = nc.dram_tensor("x16", [P, N], F16, kind="ExternalInput").ap()
    out_d = nc.dram_tensor("logits", [OUT, 1], F32, kind="ExternalOutput").ap()

    def ltile(lh8, k, m):
        # stationary L tile (k, m): lhsT[v, w] = L[k*128+v, m*128+w]
        t = lh8[m // 2]
        return t[:, ((m % 2) * NT + k) * P : ((m % 2) * NT + k + 1) * P]

    with tile.TileContext(nc) as tc:
        import contextlib

        with contextlib.ExitStack() as ctx:
            cb = ctx.enter_context(tc.tile_pool(name="cb", bufs=1))
            wb = ctx.enter_context(tc.tile_pool(name="wb", bufs=2))
            pm = ctx.enter_context(tc.tile_pool(name="pm", bufs=2, space="PSUM"))
            pz = ctx.enter_context(tc.tile_pool(name="pz", bufs=2, space="PSUM"))
            pc = ctx.enter_context(tc.tile_pool(name="pc", bufs=4, space="PSUM"))

            # ---- input loads: small packs first (feature/coef start early),
            # then X^T, then 8 L chunks as separate tiles so colsum can
            # stream behind the DMA (chunk i = m-blocks 2i, 2i+1).
            fpk = cb.tile([P, FW], F32, tag="fpk")
            nc.sync.dma_start(out=fpk[:], in_=fpk_d)
            f16pk = cb.tile([P, F16W], F16, tag="f16pk")
            nc.sync.dma_start(out=f16pk[:], in_=f16pk_d)
            x16 = cb.tile([P, N], F16, tag="x16")
            nc.sync.dma_start(out=x16[:], in_=x16_d)
            lh8 = []
            for i in range(8):
                t = cb.tile([P, NT * N // 8], F8, name=f"lh_{i}", tag=f"lh_{i}")
                nc.scalar.dma_start(out=t[:], in_=lpk_d[:, bass.ts(i, NT * N // 8)])
                lh8.append(t)

            th = fpk[0 : K + 1, 0:1]
            m2 = fpk[0 : K + 1, 1 : 1 + (K + 1)]
            w2 = fpk[0:HID, 12 : 12 + OUT]
            w1 = f16pk[:, 0:HID]
            b1row16 = f16pk[0:1, HID : HID + HID]
            ones16 = cb.tile([P, 1], F16, tag="ones16")
            nc.vector.memset(ones16[:], 1.0)
            ones16r = cb.tile([1, P], F16, tag="ones16r")
            nc.vector.memset(ones16r[:], 1.0)
            ident1 = cb.tile([1, 1], F32, tag="ident1")
            nc.vector.memset(ident1[:], 1.0)

            onesr_t = cb.tile([1, P], F32, tag="onesr")
            nc.vector.memset(onesr_t[:], 1.0)
            onesr = onesr_t[0:1, 0:P]
            b2row = fpk[0:1, 28 : 28 + OUT]

            # ---- coefficients: coefRow = theta^T @ M2 -> broadcast to 128 rows
            ps_cf = pm.tile([1, K + 1], F32, tag="pm")
            nc.tensor.matmul(ps_cf[:], th, m2, start=True, stop=True)
            cfrow = cb.tile([1, K + 1], F32, tag="cfrow")
            nc.vector.tensor_copy(cfrow[:], ps_cf[:])
            ps_cb = pm.tile([P, K + 1], F32, tag="pm")
            nc.tensor.matmul(ps_cb[:], onesr, cfrow[:], start=True, stop=True)
            coefb = cb.tile([P, K + 1], F32, tag="coefb")
            nc.vector.tensor_copy(coefb[:], ps_cb[:])

            # per-iteration mean-ledger scalars replicated per partition:
            # scale_j = A_j/N, bias_j = -A_j/N (for the colsum->d16_0 fold)
            sc0 = cb.tile([P, 1], F32, tag="sc0")
            nc.vector.tensor_scalar_mul(sc0[:], coefb[:, 0:1], INV_N)
            nb0 = cb.tile([P, 1], F32, tag="nb0")
            nc.vector.tensor_scalar_mul(nb0[:], sc0[:], -LSC)
            nbias = cb.tile([P, 1], F32, tag="nbias")
            nc.vector.memset(nbias[:], -INV_N)

            G = 4  # m-groups per chain step (psum bank + assembly granularity)
            GM = NT // G

            # ---- colsum(L): d16_0 = fp16(A_0*(colsum/N - 1/N)) per group,
            # plus eps = colsum/N - 1/N in fp32 for the later iterations.
            d16 = [wb.tile([P, GM], F16, name=f"d16i0_{g}", tag=f"d16_{g}") for g in range(G)]
            eps = cb.tile([P, NT], F32, tag="eps")
            for g in range(G):
                ps_cs = pc.tile([P, GM], F32, tag="pc")
                for mg in range(GM):
                    m = g * GM + mg
                    for k in range(NT):
                        nc.tensor.matmul(
                            ps_cs[:, mg : mg + 1],
                            ltile(lh8, k, m),
                            ones16[:],
                            start=(k == 0),
                            stop=(k == NT - 1),
                        )
                nc.scalar.activation(
                    d16[g][:], ps_cs[:], mybir.ActivationFunctionType.Identity,
                    bias=nb0[:], scale=sc0[:],
                )
                nc.scalar.activation(
                    eps[:, g * GM : (g + 1) * GM], ps_cs[:],
                    mybir.ActivationFunctionType.Identity,
                    bias=nbias[:], scale=INV_N / LSC,
                )

            # ---- feature side: Hf = relu(X W1 + b1), natural [v, h] layout
            # (X^T arrives pre-transposed; fp16 operands, fp32 PSUM accum)
            hf = cb.tile([P, NT * HID], F32, tag="hf")
            for t in range(NT):
                ps_z = pz.tile([P, HID], F32, tag="pz")
                nc.tensor.matmul(ps_z[:], x16[:, bass.ts(t, P)], w1, start=True, stop=False)
                nc.tensor.matmul(ps_z[:], ones16r[:], b1row16, start=False, stop=True)
                nc.scalar.activation(
                    hf[:, bass.ts(t, HID)], ps_z[:], mybir.ActivationFunctionType.Relu
                )

            # all A_j * eps tiles up front (off the critical path)
            epsa = []
            for it in range(1, K):
                ea = cb.tile([P, NT], F32, tag=f"epsa_{it}")
                nc.vector.tensor_scalar(ea[:], eps[:], coefb[:, it : it + 1], LSC * LSC, mybir.AluOpType.mult, mybir.AluOpType.mult)
                epsa.append(ea)

            # ---- chain: d' = L^T d + A_j * eps   (d zero-sum, fp16 storage)
            dfin = None
            for it in range(1, K):
                ea = epsa[it - 1]
                last = it == K - 1
                d16n = None if last else [
                    wb.tile([P, GM], F16, name=f"d16i{it}_{g}", tag=f"d16_{g}")
                    for g in range(G)
                ]
                if last:
                    dfin = [
                        wb.tile([P, GM], F32, name=f"dfin_{g}", tag=f"dfin_{g}")
                        for g in range(G)
                    ]
                ps_g = [
                    pc.tile([P, GM], F32, name=f"psch{it}_{g}", tag="pc")
                    for g in range(G)
                ]
                for k in range(NT):
                    rhs = d16[k // GM][:, k % GM : k % GM + 1]
                    for m in range(NT):
                        nc.tensor.matmul(
                            ps_g[m // GM][:, m % GM : m % GM + 1],
                            ltile(lh8, k, m),
                            rhs,
                            start=(k == 0),
                            stop=(k == NT - 1),
                        )
                for g in range(G):
                    tgt = dfin[g] if last else d16n[g]
                    if last:
                        # dfin stays in scaled space; descale folds into the
                        # wf activation below
                        nc.vector.tensor_add(
                            tgt[:], ps_g[g][:], ea[:, g * GM : (g + 1) * GM]
                        )
                    else:
                        tmp = wb.tile(
                            [P, GM], F32, name=f"asm{it}_{g}", tag=f"asm_{g}"
                        )
                        nc.vector.tensor_add(
                            tmp[:], ps_g[g][:], ea[:, g * GM : (g + 1) * GM]
                        )
                        nc.vector.tensor_scalar_mul(tgt[:], tmp[:], 1.0 / LSC)
                if not last:
                    d16 = d16n

            # w = (T/N) 1 + d ; s = w^T Hf  (per group, so s-matmuls of group g
            # start as soon as group g's chain output lands)
            tn = cb.tile([P, 1], F32, tag="tn")
            nc.scalar.mul(tn[:], coefb[:, K : K + 1], INV_N)
            wf = cb.tile([P, NT], F32, tag="wf")
            ps_s = pm.tile([1, HID], F32, tag="pm")
            for g in range(G):
                nc.scalar.activation(
                    wf[:, g * GM : (g + 1) * GM], dfin[g][:],
                    mybir.ActivationFunctionType.Identity, bias=tn[:],
                    scale=1.0 / (LSC * LSC),
                )
                for mg in range(GM):
                    t = g * GM + mg
                    nc.tensor.matmul(
                        ps_s[:],
                        wf[:, t : t + 1],
                        hf[:, bass.ts(t, HID)],
                        start=(t == 0),
                        stop=(t == NT - 1),
                    )
            srow = cb.tile([1, HID], F32, tag="srow")
            nc.vector.tensor_copy(srow[:], ps_s[:])
            ps_st = pm.tile([HID, 1], F32, tag="pm")
            nc.tensor.transpose(ps_st[:], srow[:], ident1[:])
            st = cb.tile([HID, 1], F32, tag="st")
            nc.vector.tensor_copy(st[:], ps_st[:])
            ps_o = pm.tile([OUT, 1], F32, tag="pm")
            nc.tensor.matmul(ps_o[:], w2, st[:], start=True, stop=False)
            nc.tensor.matmul(ps_o[:], b2row, onesr[0:1, 0:1], start=False, stop=True)
            outt = cb.tile([OUT, 1], F32, tag="outt")
            nc.vector.tensor_copy(outt[:], ps_o[:])
            nc.gpsimd.dma_start(out=out_d, in_=outt[:])

    nc.compile()
    return nc


_NC_CACHE = {}


def _get_program():
    if "nc" not in _NC_CACHE:
        _NC_CACHE["nc"] = _build_program()
    return _NC_CACHE["nc"]


def _prepare_in_maps(X, L, W1, b1, W2, b2, theta):
    import ml_dtypes
    lpk = (
        (np.ascontiguousarray(L, np.float32) * np.float32(LSC))
        .reshape(NT, P, NT, P)
        .transpose(1, 2, 0, 3)
        .reshape(P, NT * N)
        .astype(ml_dtypes.float8_e4m3)
    )
    fpk = np.zeros((P, 64), np.float32)
    fpk[0 : K + 1, 0] = np.asarray(theta, np.float32)
    fpk[0 : K + 1, 1 : 1 + (K + 1)] = _m2_matrix()
    fpk[0:HID, 12 : 12 + OUT] = np.asarray(W2, np.float32)
    fpk[0, 28 : 28 + OUT] = np.asarray(b2, np.float32)
    f16pk = np.zeros((P, 128), np.float16)
    f16pk[0:F0, 0:HID] = np.asarray(W1, np.float32).astype(np.float16)
    f16pk[0, HID : HID + HID] = np.asarray(b1, np.float32).astype(np.float16)
    common = {"lpk": lpk, "fpk": fpk, "f16pk": f16pk}
    in_maps = []
    for b in range(B):
        x16 = np.ascontiguousarray(
            np.asarray(X[b], np.float32).T.astype(np.float16)
        )
        in_maps.append({**common, "x16": x16})
    return in_maps


def _run(inputs, trace=False):
    nc = _get_program()
    in_maps = _prepare_in_maps(
        inputs["X"], inputs["L"], inputs["W1"], inputs["b1"],
        inputs["W2"], inputs["b2"], inputs["theta"],
    )
    res = run_bass_kernel_spmd(nc, in_maps, list(range(B)), trace=trace)
    out = np.stack([res.results[b]["logits"].reshape(OUT) for b in range(B)])
    return out.astype(np.float32), res


def kernel(**inputs) -> np.ndarray:
    out, _ = _run(inputs, trace=False)
    return out


def kernel_traced(**inputs):
    return _run(inputs, trace=True)
